# revision 15
# baseline (speedup 1.0000x reference)
"""Trainium2 Bass kernel for nn_AttDecoder (attention-LSTM decoder).

Strategy (8 NeuronCores, SPMD):
  - Attention is data-parallel over batch (8 samples/core, context+keys resident).
  - LSTM gates are tensor-parallel over the 4H gate dim (512 gate cols/core,
    arranged as [i|f|g|o] x 128 for this core's hidden slice). Wv is folded into
    W_ih (gates use pooled directly), removing the val matmul.
  - The [B*T, H] x [H, V] prediction matmul is vocab-sharded (4000 cols/core,
    bf16), deferred into step pairs (M=128) and interleaved into collective
    wait gaps.
  - Per step: AllGather(pooled), AllGather(h slices), AllToAll(own-sample h
    pack for next step's attention scores).
  - Matmuls in float32r (tf32-like) except pred/pooled in bf16.

kernel(**inputs) takes the FULL inputs and returns (preds, hT, cT).
"""
import os
import sys

sys.path.insert(0, "/opt/trn_rl_repo")

import numpy as np
import ml_dtypes

import concourse.bass as bass
import concourse.mybir as mybir
import concourse.tile as tile
from concourse import bacc

F32 = mybir.dt.float32
F32R = mybir.dt.float32r
BF16 = mybir.dt.bfloat16

P = 128
B, S, C, H, E, V, T = 64, 128, 512, 1024, 512, 32000, 16
R = 8            # cores
BL = B // R      # local samples per core (8)
G = 4 * H // R   # gate slice (512)
HS = H // R      # hidden slice (128)
VS = V // R      # vocab slice (4000)
VCH = 500        # vocab chunk per pred matmul (8 chunks)
NCH = VS // VCH
KT_H = H // P    # 8 h tiles
KT_C = C // P    # 4
KT_E = E // P    # 4
NSTEPS = int(os.environ.get("ATT_NSTEPS", str(T)))

Exp = mybir.ActivationFunctionType.Exp
Sig = mybir.ActivationFunctionType.Sigmoid
Tanh = mybir.ActivationFunctionType.Tanh


def build_nc(nsteps=NSTEPS):
    nc = bacc.Bacc(None, num_devices=R, target_bir_lowering=False, debug=False)

    # ---- per-core inputs --------------------------------------------------
    ctxT_in = nc.declare_dram_parameter("ctxT", [P, KT_C * BL * S], F32, isOutput=False)
    ctxflat_in = nc.declare_dram_parameter("ctxflat", [P, BL * C], BF16, isOutput=False)
    wkT_in = nc.declare_dram_parameter("wkT", [P, KT_C * H], F32, isOutput=False)
    bk_in = nc.declare_dram_parameter("bk", [P, KT_H], F32, isOutput=False)
    wcomb_in = nc.declare_dram_parameter("wcomb", [P, 16 * G], F32, isOutput=False)
    bcomb_in = nc.declare_dram_parameter("bcomb", [B, G], F32, isOutput=False)
    wp_in = nc.declare_dram_parameter("wp", [P, KT_H * VS], BF16, isOutput=False)
    bp_in = nc.declare_dram_parameter("bp", [P, VS], BF16, isOutput=False)
    embT_in = nc.declare_dram_parameter("embT", [P, T * KT_E * B], F32, isOutput=False)
    h0slots_in = nc.declare_dram_parameter("h0slots", [P, R * B], F32, isOutput=False)
    hpack0_in = nc.declare_dram_parameter("hpack0", [P, R * BL], F32, isOutput=False)
    c0T_in = nc.declare_dram_parameter("c0T", [B, HS], F32, isOutput=False)
    id64_in = nc.declare_dram_parameter("id64", [B, B], F32, isOutput=False)
    id8_in = nc.declare_dram_parameter("id8", [BL, BL], F32, isOutput=False)
    id8b_in = nc.declare_dram_parameter("id8b", [BL, BL], BF16, isOutput=False)
    mask_in = nc.declare_dram_parameter("mask", [BL, BL * S], BF16, isOutput=False)

    preds_out = nc.declare_dram_parameter("preds", [B, T, VS], F32, isOutput=True)
    h_out = nc.declare_dram_parameter("h_fin", [B, HS], F32, isOutput=True)
    c_out = nc.declare_dram_parameter("c_fin", [B, HS], F32, isOutput=True)

    rg = [list(range(R))]

    with tile.TileContext(nc) as tc:
        with (
            tc.tile_pool(name="res", bufs=1) as res,          # resident tensors
            tc.tile_pool(name="small", bufs=2) as small,      # per-step small tiles
            tc.tile_pool(name="stage", bufs=3) as stage,      # pred staging + emb bufs
            tc.tile_pool(name="pairs", bufs=2) as pairs,
            tc.tile_pool(name="attp", bufs=1) as attpool,      # bf16 state pairs
            tc.tile_pool(name="ps_big", bufs=1, space="PSUM") as ps_big,
            tc.tile_pool(name="ps_sc", bufs=1, space="PSUM") as ps_sc,
            tc.tile_pool(name="ps_small", bufs=1, space="PSUM") as ps_small,
            tc.tile_pool(name="ps_pred", bufs=2, space="PSUM") as ps_pred,
            tc.tile_pool(name="dram", bufs=4, space="DRAM") as dram,
        ):
            # ---- resident loads ------------------------------------------
            keyT = res.tile([P, KT_H * BL * S], F32R)          # 4 MB
            ctxflat = res.tile([P, BL * C], BF16)              # 1 MB
            wcomb = res.tile([P, 16 * G], F32R)                # 4 MB
            wp = res.tile([P, KT_H * VS], BF16)                # 8 MB
            bp = res.tile([P, VS], BF16)                       # 1 MB
            bcomb = res.tile([B, G], F32)
            hquad = res.tile([P, 4 * R * B], F32R)             # 1 MB state slots
            h0slots = res.tile([P, R * B], F32R)
            id64 = res.tile([B, B], F32)
            id8 = res.tile([BL, BL], F32)
            id8b = res.tile([BL, BL], BF16)
            maskb = res.tile([BL, BL * S], BF16)
            bk = res.tile([P, KT_H], F32)

            nc.sync.dma_start(ctxflat[:], ctxflat_in[:])
            nc.sync.dma_start(wcomb[:], wcomb_in[:].bitcast(F32R))
            nc.sync.dma_start(wp[:], wp_in[:])
            nc.sync.dma_start(bp[:], bp_in[:])
            nc.sync.dma_start(bcomb[:], bcomb_in[:])
            nc.sync.dma_start(h0slots[:], h0slots_in[:].bitcast(F32R))
            nc.sync.dma_start(id64[:], id64_in[:])
            nc.sync.dma_start(id8[:], id8_in[:])
            nc.sync.dma_start(id8b[:], id8b_in[:])
            nc.sync.dma_start(maskb[:], mask_in[:])
            nc.sync.dma_start(bk[:], bk_in[:])

            # ---- phase 0: key precompute (two C-halves to save SBUF) -----
            with tc.tile_pool(name="phase0", bufs=1) as p0:
                for chalf in range(2):
                    cts = (chalf * 2, chalf * 2 + 1)
                    ctxTh = p0.tile([P, 2 * BL * S], F32R, tag="p0a", name=f"ctxTh{chalf}")
                    wkTh = p0.tile([P, 2 * H], F32R, tag="p0b", name=f"wkTh{chalf}")
                    nc.sync.dma_start(
                        ctxTh[:],
                        ctxT_in[:, cts[0] * BL * S:(cts[1] + 1) * BL * S].bitcast(F32R),
                    )
                    nc.sync.dma_start(
                        wkTh[:], wkT_in[:, cts[0] * H:(cts[1] + 1) * H].bitcast(F32R)
                    )
                    for j in range(KT_H):
                        for half in range(2):
                            kps = ps_big.tile([P, 512], F32, tag="keyps")
                            for ci in range(2):
                                nc.tensor.matmul(
                                    kps[:],
                                    wkTh[:, ci * H + j * P: ci * H + (j + 1) * P],
                                    ctxTh[:, ci * BL * S + half * 512: ci * BL * S + (half + 1) * 512],
                                    start=(ci == 0), stop=(ci == 1),
                                )
                            dst = keyT[:, j * BL * S + half * 512: j * BL * S + (half + 1) * 512]
                            if chalf == 0:
                                # first half: write with bias
                                if (j + half) % 2:
                                    nc.scalar.activation(
                                        dst, kps[:], mybir.ActivationFunctionType.Identity,
                                        bias=bk[:, j: j + 1], scale=1.0,
                                    )
                                else:
                                    nc.vector.tensor_scalar_add(dst, kps[:], bk[:, j: j + 1])
                            else:
                                # second half: accumulate
                                nc.vector.tensor_tensor(
                                    dst, kps[:], dst.bitcast(F32), mybir.AluOpType.add
                                )

            # ---- state handles -------------------------------------------
            c_cur = small.tile([B, HS], F32, tag="c")
            nc.sync.dma_start(c_cur[:], c0T_in[:])
            hpack = res.tile([P, R * BL], F32R)                # own-sample pack
            nc.sync.dma_start(hpack[:], hpack0_in[:].bitcast(F32R))

            pending_preds = []  # list of emitters

            def emit_pred_chunks(n):
                for _ in range(n):
                    if pending_preds:
                        pending_preds.pop(0)()

            def step(t, state_tile, state_off):
                """Emit one decoder step. state_tile[:, state_off:state_off+R*B]
                holds state_t slot-major. Returns nothing; updates c_cur/hpack,
                writes state_{t+1} into hquad slot t%4, enqueues preds."""
                nonlocal c_cur

                # --- gates h-term (8 MMs, start group) ---
                gps = ps_big.tile([B, G], F32, tag="gatesps")
                for j in range(KT_H):
                    nc.tensor.matmul(
                        gps[:],
                        state_tile[:, state_off + j * B: state_off + (j + 1) * B],
                        wcomb[:, (8 + j) * G: (9 + j) * G],
                        start=(j == 0), stop=False, skip_group_check=True,
                    )

                # --- emb prefetch for this step ---
                emb_sb = stage.tile([P, KT_E * B], F32R, tag="emb")
                nc.sync.dma_start(
                    emb_sb[:], embT_in[:, t * KT_E * B: (t + 1) * KT_E * B].bitcast(F32R)
                )

                # --- scores (16 MMs) + diag mask (2 MMs) ---
                scps = ps_sc.tile([BL, BL * S], F32, tag="scps")
                for j in range(KT_H):
                    for half in range(2):
                        nc.tensor.matmul(
                            scps[:, half * 512:(half + 1) * 512],
                            hpack[:, j * BL:(j + 1) * BL],
                            keyT[:, j * BL * S + half * 512: j * BL * S + (half + 1) * 512],
                            start=(j == 0), stop=False,
                            skip_group_check=True,
                        )
                for half in range(2):
                    nc.tensor.matmul(
                        scps[:, half * 512:(half + 1) * 512],
                        id8b[:], maskb[:, half * 512:(half + 1) * 512],
                        start=False, stop=(half == 1), skip_group_check=True,
                    )

                emit_pred_chunks(2)

                # --- softmax on masked rows [8, 1024] ---
                negmax = small.tile([BL, 1], F32, tag="negmax")
                nc.vector.tensor_reduce(
                    out=negmax[:], in_=scps[:], op=mybir.AluOpType.max,
                    axis=mybir.AxisListType.X, negate=True,
                )
                sumexp = small.tile([BL, 1], F32, tag="sumexp")
                nc.scalar.activation(scps[:], scps[:], Exp, bias=negmax[:], scale=1.0,
                                     accum_out=sumexp[:])
                rec = small.tile([BL, 1], F32, tag="rec")
                nc.vector.reciprocal(rec[:], sumexp[:])
                att = attpool.tile([BL, BL * S], F32, tag="att", name=f"att{t}")
                nc.vector.tensor_scalar_mul(att[:], scps[:], rec[:])

                # --- per-block transposes -> attFLAT (masked zeros elsewhere) ---
                attflat = small.tile([P, BL * BL], BF16, tag="attflat")
                for kt in range(BL):
                    atps = ps_small.tile([S, BL], F32, tag="tiny", name=f"atps{kt}")
                    nc.tensor.transpose(atps[:], att[:, kt * S:(kt + 1) * S], id8[:])
                    nc.vector.tensor_copy(attflat[:, kt * BL:(kt + 1) * BL], atps[:])

                # --- pooledT (32 MMs) ---
                pps = ps_small.tile([P, KT_C * BL], F32, tag="tiny")
                for ct in range(KT_C):
                    for kt in range(BL):
                        nc.tensor.matmul(
                            pps[:, ct * BL:(ct + 1) * BL],
                            ctxflat[:, kt * C + ct * P: kt * C + (ct + 1) * P],
                            attflat[:, kt * BL:(kt + 1) * BL],
                            start=(kt == 0), stop=(kt == BL - 1),
                            skip_group_check=True,
                        )
                pool_sb = small.tile([P, KT_C * BL], F32, tag="poolsb")
                nc.vector.tensor_copy(pool_sb[:], pps[:])

                # --- AG#1 pooled ---
                agp_in = dram.tile([P, KT_C * BL], F32, tag="agp_in")
                agp_out = dram.tile([R * P, KT_C * BL], F32, tag="agp_out")
                nc.sync.dma_start(agp_in[:], pool_sb[:])
                nc.gpsimd.collective_compute(
                    "AllGather", mybir.AluOpType.bypass, replica_groups=rg,
                    ins=[agp_in[:].opt()], outs=[agp_out[:].opt()],
                )
                poolfull = small.tile([P, KT_C, R, BL], F32R, tag="poolfull")
                nc.sync.dma_start(
                    poolfull[:],
                    agp_out[:].rearrange("(k p) (ct bl) -> p ct k bl", p=P, ct=KT_C).bitcast(F32R),
                )

                emit_pred_chunks(2)

                # --- gates emb + pooled terms ---
                for et in range(KT_E):
                    nc.tensor.matmul(
                        gps[:], emb_sb[:, et * B:(et + 1) * B],
                        wcomb[:, et * G:(et + 1) * G],
                        start=False, stop=False, skip_group_check=True,
                    )
                for ct in range(KT_C):
                    nc.tensor.matmul(
                        gps[:],
                        poolfull[:, ct].rearrange("p k bl -> p (k bl)"),
                        wcomb[:, (4 + ct) * G: (5 + ct) * G],
                        start=False, stop=(ct == KT_C - 1), skip_group_check=True,
                    )

                # --- bias + LSTM ---
                gates = small.tile([B, G], F32, tag="gates")
                nc.vector.tensor_tensor(gates[:], gps[:], bcomb[:], mybir.AluOpType.add)
                sig_i = small.tile([B, HS], F32, tag="sigi")
                sig_f = small.tile([B, HS], F32, tag="sigf")
                tanh_g = small.tile([B, HS], F32, tag="tanhg")
                sig_o = small.tile([B, HS], F32, tag="sigo")
                nc.scalar.activation(sig_i[:], gates[:, 0 * HS:1 * HS], Sig)
                nc.scalar.activation(sig_f[:], gates[:, 1 * HS:2 * HS], Sig)
                nc.scalar.activation(tanh_g[:], gates[:, 2 * HS:3 * HS], Tanh)
                nc.scalar.activation(sig_o[:], gates[:, 3 * HS:4 * HS], Sig)
                t1 = small.tile([B, HS], F32, tag="t1")
                nc.vector.tensor_tensor(t1[:], sig_f[:], c_cur[:], mybir.AluOpType.mult)
                t2 = small.tile([B, HS], F32, tag="t2")
                nc.vector.tensor_tensor(t2[:], sig_i[:], tanh_g[:], mybir.AluOpType.mult)
                c_new = small.tile([B, HS], F32, tag="c")
                nc.vector.tensor_tensor(c_new[:], t1[:], t2[:], mybir.AluOpType.add)
                tanh_c = small.tile([B, HS], F32, tag="tanhc")
                nc.scalar.activation(tanh_c[:], c_new[:], Tanh)
                h_newT = small.tile([B, HS], F32, tag="hnewT")
                nc.vector.tensor_tensor(h_newT[:], sig_o[:], tanh_c[:], mybir.AluOpType.mult)
                c_cur = c_new

                # --- h transpose ---
                hps = ps_small.tile([HS, B], F32, tag="tiny")
                nc.tensor.transpose(hps[:], h_newT[:], id64[:])
                h_slice = small.tile([HS, B], F32, tag="hslice")
                nc.vector.tensor_copy(h_slice[:], hps[:])

                # --- A2A (hpack for next step) ---
                a2a_in = dram.tile([R, P * BL], F32, tag="a2a_in")
                a2a_out = dram.tile([R, P * BL], F32, tag="a2a_out")
                nc.sync.dma_start(
                    a2a_in[:].rearrange("j (p bl) -> p j bl", p=P),
                    h_slice[:].rearrange("p (j bl) -> p j bl", j=R),
                )
                nc.gpsimd.collective_compute(
                    "AllToAll", mybir.AluOpType.bypass, replica_groups=rg,
                    ins=[a2a_in[:].opt()], outs=[a2a_out[:].opt()],
                )
                nc.sync.dma_start(
                    hpack[:].rearrange("p (j bl) -> p j bl", j=R),
                    a2a_out[:].rearrange("j (p bl) -> p j bl", p=P).bitcast(F32R),
                )

                # --- AG#2 h slices -> state_{t+1} in hquad slot t%4 ---
                agh_in = dram.tile([HS, B], F32, tag="agh_in")
                agh_out = dram.tile([R * HS, B], F32, tag="agh_out")
                nc.sync.dma_start(agh_in[:], h_slice[:])
                nc.gpsimd.collective_compute(
                    "AllGather", mybir.AluOpType.bypass, replica_groups=rg,
                    ins=[agh_in[:].opt()], outs=[agh_out[:].opt()],
                )
                q = t % 4
                nc.sync.dma_start(
                    hquad[:, q * R * B:(q + 1) * R * B].rearrange("p (j b) -> p j b", j=R),
                    agh_out[:].rearrange("(j p) b -> p j b", p=P).bitcast(F32R),
                )

                # bf16 copy for pred (state s = t+1, pair pos = t % 2)
                if t % 2 == 0:
                    sbf = pairs.tile([P, 2 * R * B], BF16, tag="pairbuf", name=f"pair_{t}")
                else:
                    sbf = None
                return c_new, h_newT, q, sbf

            # ---- main loop -----------------------------------------------
            state_tile, state_off = h0slots, 0
            cur_pair = None
            for t in range(nsteps):
                c_new, h_newT, q, new_pair = step(t, state_tile, state_off)
                if new_pair is not None:
                    cur_pair = new_pair
                # copy state_{t+1} bf16 into pair position t%2
                nc.vector.tensor_copy(
                    cur_pair[:].rearrange("p (j d b) -> p j d b", j=R, d=2)[:, :, t % 2, :],
                    hquad[:, q * R * B:(q + 1) * R * B].rearrange("p (j b) -> p j b", j=R).bitcast(F32),
                )
                state_tile, state_off = hquad, q * R * B

                if t % 2 == 1:
                    # pair (t-1, t) complete -> enqueue pred chunks
                    pair_tile = cur_pair
                    p_idx = t // 2
                    for nchk in range(NCH):
                        def emit(pair_tile=pair_tile, p_idx=p_idx, nchk=nchk):
                            pps = ps_pred.tile([P, VCH], F32, tag="predps")
                            for j in range(KT_H):
                                nc.tensor.matmul(
                                    pps[:],
                                    pair_tile[:, j * 2 * B:(j + 1) * 2 * B],
                                    wp[:, j * VS + nchk * VCH: j * VS + (nchk + 1) * VCH],
                                    start=(j == 0), stop=(j == KT_H - 1),
                                    skip_group_check=True,
                                )
                            stg = stage.tile([P, VCH], F32, tag="predstage")
                            nc.vector.tensor_tensor(
                                stg[:], pps[:], bp[:, nchk * VCH:(nchk + 1) * VCH],
                                mybir.AluOpType.add,
                            )
                            for d in range(2):
                                nc.sync.dma_start(
                                    preds_out[:, 2 * p_idx + d, nchk * VCH:(nchk + 1) * VCH],
                                    stg[d * B:(d + 1) * B, :],
                                )
                        pending_preds.append(emit)

                if t == nsteps - 1:
                    nc.sync.dma_start(h_out[:], h_newT[:])
                    nc.sync.dma_start(c_out[:], c_new[:])

            emit_pred_chunks(len(pending_preds))

    nc.finalize()
    return nc


# ---------------------------------------------------------------------------
# Host side
# ---------------------------------------------------------------------------

def _host_prep(inputs):
    ctx = np.asarray(inputs["context"], np.float32)      # [B, S, C]
    data = np.asarray(inputs["data"])                    # [B, T]
    h0 = np.asarray(inputs["h0"], np.float32)            # [B, H]
    c0 = np.asarray(inputs["c0"], np.float32)
    Wk = np.asarray(inputs["Wk"], np.float32)            # [H, C]
    bk = np.asarray(inputs["bk"], np.float32)
    Wv = np.asarray(inputs["Wv"], np.float32)            # [H, C]
    bv = np.asarray(inputs["bv"], np.float32)
    embed = np.asarray(inputs["embed"], np.float32)      # [V, E]
    W_ih = np.asarray(inputs["W_ih"], np.float32)        # [4H, E+H]
    W_hh = np.asarray(inputs["W_hh"], np.float32)        # [4H, H]
    b_ih = np.asarray(inputs["b_ih"], np.float32)
    b_hh = np.asarray(inputs["b_hh"], np.float32)
    Wp = np.asarray(inputs["Wp"], np.float32)            # [V, H]
    bp = np.asarray(inputs["bp"], np.float32)

    W_ih_emb = W_ih[:, :E]                               # [4H, E]
    W_ih_val = W_ih[:, E:]                               # [4H, H]
    W_fold = W_ih_val @ Wv                               # [4H, C]
    b_comb = b_ih + b_hh + W_ih_val @ bv                 # [4H]
    Kmat = np.concatenate([W_ih_emb.T, W_fold.T, W_hh.T], axis=0)  # [2048, 4H]

    emb_all = embed[data]                                # [B, T, E]
    embT = np.ascontiguousarray(
        emb_all.transpose(2, 1, 0).reshape(KT_E, P, T, B).transpose(1, 2, 0, 3)
    ).reshape(P, T * KT_E * B)

    h0slots = np.ascontiguousarray(
        h0.T.reshape(R, P, B).transpose(1, 0, 2)
    ).reshape(P, R * B)

    bk_t = np.ascontiguousarray(bk.reshape(KT_H, P).T)

    mask = np.full((BL, BL * S), -1e30, np.float32)
    for b in range(BL):
        mask[b, b * S:(b + 1) * S] = 0.0

    in_maps = []
    for r in range(R):
        sl = slice(BL * r, BL * (r + 1))
        ctx_r = ctx[sl]                                  # [8, S, C]
        ctxT = np.ascontiguousarray(
            ctx_r.transpose(2, 0, 1).reshape(KT_C, P, BL * S).transpose(1, 0, 2)
        ).reshape(P, KT_C * BL * S)
        ctxflat = np.ascontiguousarray(
            ctx_r.transpose(1, 0, 2).reshape(P, BL * C)
        ).astype(ml_dtypes.bfloat16)
        wkT = np.ascontiguousarray(
            Wk.T.reshape(KT_C, P, H).transpose(1, 0, 2)
        ).reshape(P, KT_C * H)

        g_idx = np.concatenate([np.arange(HS) + g * H + HS * r for g in range(4)])
        wcomb = np.ascontiguousarray(
            Kmat[:, g_idx].reshape(16, P, G).transpose(1, 0, 2)
        ).reshape(P, 16 * G)
        bcomb = np.tile(b_comb[g_idx][None, :], (B, 1)).astype(np.float32)

        vsl = slice(VS * r, VS * (r + 1))
        wp_r = np.ascontiguousarray(
            Wp[vsl].T.reshape(KT_H, P, VS).transpose(1, 0, 2)
        ).reshape(P, KT_H * VS).astype(ml_dtypes.bfloat16)
        bp_r = np.tile(bp[vsl][None, :], (P, 1)).astype(ml_dtypes.bfloat16)

        hpack0 = np.ascontiguousarray(
            h0slots.reshape(P, R, B)[:, :, BL * r: BL * (r + 1)]
        ).reshape(P, R * BL)

        in_maps.append({
            "ctxT": ctxT,
            "ctxflat": ctxflat,
            "wkT": wkT,
            "bk": bk_t,
            "wcomb": wcomb,
            "bcomb": bcomb,
            "wp": wp_r,
            "bp": bp_r,
            "embT": embT,
            "h0slots": h0slots,
            "hpack0": hpack0,
            "c0T": np.ascontiguousarray(c0[:, HS * r: HS * (r + 1)]),
            "id64": np.eye(B, dtype=np.float32),
            "id8": np.eye(BL, dtype=np.float32),
            "id8b": np.eye(BL, dtype=ml_dtypes.bfloat16),
            "mask": mask.astype(ml_dtypes.bfloat16),
        })
    return in_maps


_CACHE = {}


def _get_runner():
    if "runner" in _CACHE:
        return _CACHE["runner"]
    import jax
    from jax.sharding import Mesh, PartitionSpec
    from jax.experimental.shard_map import shard_map
    from concourse.bass2jax import _bass_exec_p, install_neuronx_cc_hook, partition_id_tensor

    nc = build_nc()
    install_neuronx_cc_hook()
    partition_name = nc.partition_id_tensor.name if nc.partition_id_tensor else None

    in_names, out_names, out_avals, zero_outs = [], [], [], []
    for alloc in nc.m.functions[0].allocations:
        if not isinstance(alloc, mybir.MemoryLocationSet):
            continue
        name = alloc.memorylocations[0].name
        if alloc.kind == "ExternalInput":
            if name != partition_name:
                in_names.append(name)
        elif alloc.kind == "ExternalOutput":
            out_names.append(name)
            shape = tuple(alloc.tensor_shape)
            dtype = mybir.dt.np(alloc.dtype)
            out_avals.append(jax.core.ShapedArray(shape, dtype))
            zero_outs.append(np.zeros(shape, dtype))
    n_params = len(in_names)
    n_outs = len(out_avals)
    all_in_names = list(in_names) + list(out_names)
    if partition_name is not None:
        all_in_names.append(partition_name)

    def _body(*args):
        operands = list(args)
        if partition_name is not None:
            operands.append(partition_id_tensor())
        outs = _bass_exec_p.bind(
            *operands,
            out_avals=tuple(out_avals),
            in_names=tuple(all_in_names),
            out_names=tuple(out_names),
            lowering_input_output_aliases=(),
            sim_require_finite=True,
            sim_require_nnan=True,
            nc=nc,
        )
        return tuple(outs)

    devices = jax.devices()[:R]
    mesh = Mesh(np.asarray(devices), ("core",))
    in_specs = (PartitionSpec("core"),) * (n_params + n_outs)
    out_specs = (PartitionSpec("core"),) * n_outs
    fn = jax.jit(
        shard_map(_body, mesh=mesh, in_specs=in_specs, out_specs=out_specs, check_rep=False),
        keep_unused=True,
    )
    sharding = jax.sharding.NamedSharding(mesh, PartitionSpec("core"))

    def run(in_maps):
        arrs = [
            jax.device_put(
                np.concatenate([np.asarray(m[name]) for m in in_maps], axis=0), sharding
            )
            for name in in_names
        ]
        zeros = [
            jax.device_put(np.zeros((R * z.shape[0], *z.shape[1:]), z.dtype), sharding)
            for z in zero_outs
        ]
        out_arrs = fn(*arrs, *zeros)
        res = []
        for c in range(R):
            d = {}
            for i, name in enumerate(out_names):
                full = np.asarray(out_arrs[i])
                d[name] = full.reshape(R, *out_avals[i].shape)[c]
            res.append(d)
        return res

    _CACHE["runner"] = run
    return run


def kernel(**inputs):
    in_maps = _host_prep(inputs)
    run = _get_runner()
    res = run(in_maps)
    preds = np.concatenate([res[r]["preds"] for r in range(R)], axis=2)
    hT = np.concatenate([res[r]["h_fin"] for r in range(R)], axis=1)
    cT = np.concatenate([res[r]["c_fin"] for r in range(R)], axis=1)
    return preds, hT, cT


# revision 16
# speedup vs baseline: 112.8295x; 112.8295x over previous
"""Trainium2 Bass kernel for nn_AttDecoder (attention-LSTM decoder).

Strategy (8 NeuronCores, SPMD):
  - Attention is data-parallel over batch (8 samples/core, context+keys resident).
  - LSTM gates are tensor-parallel over the 4H gate dim (512 gate cols/core,
    arranged as [i|f|g|o] x 128 for this core's hidden slice). Wv is folded into
    W_ih (gates use pooled directly), removing the val matmul.
  - The [B*T, H] x [H, V] prediction matmul is vocab-sharded (4000 cols/core,
    bf16), deferred into step pairs (M=128) and interleaved into collective
    wait gaps.
  - Per step: AllGather(pooled), AllGather(h slices), AllToAll(own-sample h
    pack for next step's attention scores).
  - Matmuls in float32r (tf32-like) except pred/pooled in bf16.

kernel(**inputs) takes the FULL inputs and returns (preds, hT, cT).
"""
import os
import sys

sys.path.insert(0, "/opt/trn_rl_repo")

import numpy as np
import ml_dtypes

import concourse.bass as bass
import concourse.mybir as mybir
import concourse.tile as tile
from concourse import bacc

F32 = mybir.dt.float32
F32R = mybir.dt.float32r
BF16 = mybir.dt.bfloat16

P = 128
B, S, C, H, E, V, T = 64, 128, 512, 1024, 512, 32000, 16
R = 8            # cores
BL = B // R      # local samples per core (8)
G = 4 * H // R   # gate slice (512)
HS = H // R      # hidden slice (128)
VS = V // R      # vocab slice (4000)
VCH = 500        # vocab chunk per pred matmul (8 chunks)
NCH = VS // VCH
KT_H = H // P    # 8 h tiles
KT_C = C // P    # 4
KT_E = E // P    # 4
NSTEPS = int(os.environ.get("ATT_NSTEPS", str(T)))

Exp = mybir.ActivationFunctionType.Exp
Sig = mybir.ActivationFunctionType.Sigmoid
Tanh = mybir.ActivationFunctionType.Tanh


def build_nc(nsteps=NSTEPS):
    nc = bacc.Bacc(None, num_devices=R, target_bir_lowering=False, debug=False)

    # ---- per-core inputs --------------------------------------------------
    ctxT_in = nc.declare_dram_parameter("ctxT", [P, KT_C * BL * S], F32, isOutput=False)
    ctxflat_in = nc.declare_dram_parameter("ctxflat", [P, BL * C], BF16, isOutput=False)
    wkT_in = nc.declare_dram_parameter("wkT", [P, KT_C * H], F32, isOutput=False)
    bk_in = nc.declare_dram_parameter("bk", [P, KT_H], F32, isOutput=False)
    wcomb_in = nc.declare_dram_parameter("wcomb", [P, 16 * G], F32, isOutput=False)
    bcomb_in = nc.declare_dram_parameter("bcomb", [B, G], F32, isOutput=False)
    wp_in = nc.declare_dram_parameter("wp", [P, KT_H * VS], BF16, isOutput=False)
    bp_in = nc.declare_dram_parameter("bp", [P, VS], BF16, isOutput=False)
    embT_in = nc.declare_dram_parameter("embT", [P, T * KT_E * B], F32, isOutput=False)
    h0slots_in = nc.declare_dram_parameter("h0slots", [P, R * B], F32, isOutput=False)
    hpack0_in = nc.declare_dram_parameter("hpack0", [P, R * BL], F32, isOutput=False)
    c0T_in = nc.declare_dram_parameter("c0T", [B, HS], F32, isOutput=False)
    id64_in = nc.declare_dram_parameter("id64", [B, B], F32, isOutput=False)
    id8_in = nc.declare_dram_parameter("id8", [BL, BL], F32, isOutput=False)
    id8b_in = nc.declare_dram_parameter("id8b", [BL, BL], BF16, isOutput=False)
    mask_in = nc.declare_dram_parameter("mask", [BL, BL * S], BF16, isOutput=False)

    preds_out = nc.declare_dram_parameter("preds", [B, T, VS], F32, isOutput=True)
    h_out = nc.declare_dram_parameter("h_fin", [B, HS], F32, isOutput=True)
    c_out = nc.declare_dram_parameter("c_fin", [B, HS], F32, isOutput=True)

    rg = [list(range(R))]

    with tile.TileContext(nc) as tc:
        with (
            tc.tile_pool(name="res", bufs=1) as res,          # resident tensors
            tc.tile_pool(name="small", bufs=2) as small,      # per-step small tiles
            tc.tile_pool(name="stage", bufs=3) as stage,      # pred staging + emb bufs
            tc.tile_pool(name="pairs", bufs=2) as pairs,
            tc.tile_pool(name="attp", bufs=1) as attpool,      # bf16 state pairs
            tc.tile_pool(name="ps_big", bufs=1, space="PSUM") as ps_big,
            tc.tile_pool(name="ps_sc", bufs=1, space="PSUM") as ps_sc,
            tc.tile_pool(name="ps_small", bufs=1, space="PSUM") as ps_small,
            tc.tile_pool(name="ps_pred", bufs=2, space="PSUM") as ps_pred,
            tc.tile_pool(name="dram", bufs=4, space="DRAM") as dram,
        ):
            # ---- resident loads ------------------------------------------
            keyT = res.tile([P, KT_H * BL * S], F32R)          # 4 MB
            ctxflat = res.tile([P, BL * C], BF16)              # 1 MB
            wcomb = res.tile([P, 16 * G], F32R)                # 4 MB
            wp = res.tile([P, KT_H * VS], BF16)                # 8 MB
            bp = res.tile([P, VS], BF16)                       # 1 MB
            bcomb = res.tile([B, G], F32)
            hquad = res.tile([P, 4 * R * B], F32R)             # 1 MB state slots
            h0slots = res.tile([P, R * B], F32R)
            id64 = res.tile([B, B], F32)
            id8 = res.tile([BL, BL], F32)
            id8b = res.tile([BL, BL], BF16)
            maskb = res.tile([BL, BL * S], BF16)
            bk = res.tile([P, KT_H], F32)

            nc.sync.dma_start(ctxflat[:], ctxflat_in[:])
            nc.sync.dma_start(wcomb[:], wcomb_in[:].bitcast(F32R))
            nc.sync.dma_start(wp[:], wp_in[:])
            nc.sync.dma_start(bp[:], bp_in[:])
            nc.sync.dma_start(bcomb[:], bcomb_in[:])
            nc.sync.dma_start(h0slots[:], h0slots_in[:].bitcast(F32R))
            nc.sync.dma_start(id64[:], id64_in[:])
            nc.sync.dma_start(id8[:], id8_in[:])
            nc.sync.dma_start(id8b[:], id8b_in[:])
            nc.sync.dma_start(maskb[:], mask_in[:])
            nc.sync.dma_start(bk[:], bk_in[:])

            # ---- phase 0: key precompute (two C-halves to save SBUF) -----
            with tc.tile_pool(name="phase0", bufs=1) as p0:
                for chalf in range(2):
                    cts = (chalf * 2, chalf * 2 + 1)
                    ctxTh = p0.tile([P, 2 * BL * S], F32R, tag="p0a", name=f"ctxTh{chalf}")
                    wkTh = p0.tile([P, 2 * H], F32R, tag="p0b", name=f"wkTh{chalf}")
                    nc.sync.dma_start(
                        ctxTh[:],
                        ctxT_in[:, cts[0] * BL * S:(cts[1] + 1) * BL * S].bitcast(F32R),
                    )
                    nc.sync.dma_start(
                        wkTh[:], wkT_in[:, cts[0] * H:(cts[1] + 1) * H].bitcast(F32R)
                    )
                    for j in range(KT_H):
                        for half in range(2):
                            kps = ps_big.tile([P, 512], F32, tag="keyps")
                            for ci in range(2):
                                nc.tensor.matmul(
                                    kps[:],
                                    wkTh[:, ci * H + j * P: ci * H + (j + 1) * P],
                                    ctxTh[:, ci * BL * S + half * 512: ci * BL * S + (half + 1) * 512],
                                    start=(ci == 0), stop=(ci == 1),
                                )
                            dst = keyT[:, j * BL * S + half * 512: j * BL * S + (half + 1) * 512]
                            if chalf == 0:
                                # first half: write with bias
                                if (j + half) % 2:
                                    nc.scalar.activation(
                                        dst, kps[:], mybir.ActivationFunctionType.Identity,
                                        bias=bk[:, j: j + 1], scale=1.0,
                                    )
                                else:
                                    nc.vector.tensor_scalar_add(dst, kps[:], bk[:, j: j + 1])
                            else:
                                # second half: accumulate
                                nc.vector.tensor_tensor(
                                    dst, kps[:], dst.bitcast(F32), mybir.AluOpType.add
                                )

            # ---- state handles -------------------------------------------
            c_cur = small.tile([B, HS], F32, tag="c")
            nc.sync.dma_start(c_cur[:], c0T_in[:])
            hpack = res.tile([P, R * BL], F32R)                # own-sample pack
            nc.sync.dma_start(hpack[:], hpack0_in[:].bitcast(F32R))

            pending_preds = []  # list of emitters

            def emit_pred_chunks(n):
                for _ in range(n):
                    if pending_preds:
                        pending_preds.pop(0)()

            def step(t, state_tile, state_off):
                """Emit one decoder step. state_tile[:, state_off:state_off+R*B]
                holds state_t slot-major. Returns nothing; updates c_cur/hpack,
                writes state_{t+1} into hquad slot t%4, enqueues preds."""
                nonlocal c_cur

                # --- gates h-term (8 MMs, start group) ---
                gps = ps_big.tile([B, G], F32, tag="gatesps")
                for j in range(KT_H):
                    nc.tensor.matmul(
                        gps[:],
                        state_tile[:, state_off + j * B: state_off + (j + 1) * B],
                        wcomb[:, (8 + j) * G: (9 + j) * G],
                        start=(j == 0), stop=False, skip_group_check=True,
                    )

                # --- emb prefetch for this step ---
                emb_sb = stage.tile([P, KT_E * B], F32R, tag="emb")
                nc.sync.dma_start(
                    emb_sb[:], embT_in[:, t * KT_E * B: (t + 1) * KT_E * B].bitcast(F32R)
                )

                # --- scores (16 MMs) + diag mask (2 MMs) ---
                scps = ps_sc.tile([BL, BL * S], F32, tag="scps")
                for j in range(KT_H):
                    for half in range(2):
                        nc.tensor.matmul(
                            scps[:, half * 512:(half + 1) * 512],
                            hpack[:, j * BL:(j + 1) * BL],
                            keyT[:, j * BL * S + half * 512: j * BL * S + (half + 1) * 512],
                            start=(j == 0), stop=False,
                            skip_group_check=True,
                        )
                for half in range(2):
                    nc.tensor.matmul(
                        scps[:, half * 512:(half + 1) * 512],
                        id8b[:], maskb[:, half * 512:(half + 1) * 512],
                        start=False, stop=(half == 1), skip_group_check=True,
                    )

                emit_pred_chunks(2)

                # --- softmax on masked rows [8, 1024] ---
                negmax = small.tile([BL, 1], F32, tag="negmax")
                nc.vector.tensor_reduce(
                    out=negmax[:], in_=scps[:], op=mybir.AluOpType.max,
                    axis=mybir.AxisListType.X, negate=True,
                )
                sumexp = small.tile([BL, 1], F32, tag="sumexp")
                nc.scalar.activation(scps[:], scps[:], Exp, bias=negmax[:], scale=1.0,
                                     accum_out=sumexp[:])
                rec = small.tile([BL, 1], F32, tag="rec")
                nc.vector.reciprocal(rec[:], sumexp[:])
                att = attpool.tile([BL, BL * S], F32, tag="att", name=f"att{t}")
                nc.vector.tensor_scalar_mul(att[:], scps[:], rec[:])

                # --- per-block transposes -> attFLAT (masked zeros elsewhere) ---
                attflat = small.tile([P, BL * BL], BF16, tag="attflat")
                for kt in range(BL):
                    atps = ps_small.tile([S, BL], F32, tag="tiny", name=f"atps{kt}")
                    nc.tensor.transpose(atps[:], att[:, kt * S:(kt + 1) * S], id8[:])
                    nc.vector.tensor_copy(attflat[:, kt * BL:(kt + 1) * BL], atps[:])

                # --- pooledT (32 MMs) ---
                pps = ps_small.tile([P, KT_C * BL], F32, tag="tiny")
                for ct in range(KT_C):
                    for kt in range(BL):
                        nc.tensor.matmul(
                            pps[:, ct * BL:(ct + 1) * BL],
                            ctxflat[:, kt * C + ct * P: kt * C + (ct + 1) * P],
                            attflat[:, kt * BL:(kt + 1) * BL],
                            start=(kt == 0), stop=(kt == BL - 1),
                            skip_group_check=True,
                        )
                pool_sb = small.tile([P, KT_C * BL], F32, tag="poolsb")
                nc.vector.tensor_copy(pool_sb[:], pps[:])

                # --- AG#1 pooled ---
                agp_in = dram.tile([P, KT_C * BL], F32, tag="agp_in")
                agp_out = dram.tile([R * P, KT_C * BL], F32, tag="agp_out")
                nc.sync.dma_start(agp_in[:], pool_sb[:])
                nc.gpsimd.collective_compute(
                    "AllGather", mybir.AluOpType.bypass, replica_groups=rg,
                    ins=[agp_in[:].opt()], outs=[agp_out[:].opt()],
                )
                poolfull = small.tile([P, KT_C, R, BL], F32R, tag="poolfull")
                nc.sync.dma_start(
                    poolfull[:],
                    agp_out[:].rearrange("(k p) (ct bl) -> p ct k bl", p=P, ct=KT_C).bitcast(F32R),
                )

                emit_pred_chunks(2)

                # --- gates emb + pooled terms ---
                for et in range(KT_E):
                    nc.tensor.matmul(
                        gps[:], emb_sb[:, et * B:(et + 1) * B],
                        wcomb[:, et * G:(et + 1) * G],
                        start=False, stop=False, skip_group_check=True,
                    )
                for ct in range(KT_C):
                    nc.tensor.matmul(
                        gps[:],
                        poolfull[:, ct].rearrange("p k bl -> p (k bl)"),
                        wcomb[:, (4 + ct) * G: (5 + ct) * G],
                        start=False, stop=(ct == KT_C - 1), skip_group_check=True,
                    )

                # --- bias + LSTM ---
                gates = small.tile([B, G], F32, tag="gates")
                nc.vector.tensor_tensor(gates[:], gps[:], bcomb[:], mybir.AluOpType.add)
                sig_i = small.tile([B, HS], F32, tag="sigi")
                sig_f = small.tile([B, HS], F32, tag="sigf")
                tanh_g = small.tile([B, HS], F32, tag="tanhg")
                sig_o = small.tile([B, HS], F32, tag="sigo")
                nc.scalar.activation(sig_i[:], gates[:, 0 * HS:1 * HS], Sig)
                nc.scalar.activation(sig_f[:], gates[:, 1 * HS:2 * HS], Sig)
                nc.scalar.activation(tanh_g[:], gates[:, 2 * HS:3 * HS], Tanh)
                nc.scalar.activation(sig_o[:], gates[:, 3 * HS:4 * HS], Sig)
                t1 = small.tile([B, HS], F32, tag="t1")
                nc.vector.tensor_tensor(t1[:], sig_f[:], c_cur[:], mybir.AluOpType.mult)
                t2 = small.tile([B, HS], F32, tag="t2")
                nc.vector.tensor_tensor(t2[:], sig_i[:], tanh_g[:], mybir.AluOpType.mult)
                c_new = small.tile([B, HS], F32, tag="c")
                nc.vector.tensor_tensor(c_new[:], t1[:], t2[:], mybir.AluOpType.add)
                tanh_c = small.tile([B, HS], F32, tag="tanhc")
                nc.scalar.activation(tanh_c[:], c_new[:], Tanh)
                h_newT = small.tile([B, HS], F32, tag="hnewT")
                nc.vector.tensor_tensor(h_newT[:], sig_o[:], tanh_c[:], mybir.AluOpType.mult)
                c_cur = c_new

                # --- h transpose ---
                hps = ps_small.tile([HS, B], F32, tag="tiny")
                nc.tensor.transpose(hps[:], h_newT[:], id64[:])
                h_slice = small.tile([HS, B], F32, tag="hslice")
                nc.vector.tensor_copy(h_slice[:], hps[:])

                # --- A2A (hpack for next step) ---
                a2a_in = dram.tile([R, P * BL], F32, tag="a2a_in")
                a2a_out = dram.tile([R, P * BL], F32, tag="a2a_out")
                nc.sync.dma_start(
                    a2a_in[:].rearrange("j (p bl) -> p j bl", p=P),
                    h_slice[:].rearrange("p (j bl) -> p j bl", j=R),
                )
                nc.gpsimd.collective_compute(
                    "AllToAll", mybir.AluOpType.bypass, replica_groups=rg,
                    ins=[a2a_in[:].opt()], outs=[a2a_out[:].opt()],
                )
                nc.sync.dma_start(
                    hpack[:].rearrange("p (j bl) -> p j bl", j=R),
                    a2a_out[:].rearrange("j (p bl) -> p j bl", p=P).bitcast(F32R),
                )

                # --- AG#2 h slices -> state_{t+1} in hquad slot t%4 ---
                agh_in = dram.tile([HS, B], F32, tag="agh_in")
                agh_out = dram.tile([R * HS, B], F32, tag="agh_out")
                nc.sync.dma_start(agh_in[:], h_slice[:])
                nc.gpsimd.collective_compute(
                    "AllGather", mybir.AluOpType.bypass, replica_groups=rg,
                    ins=[agh_in[:].opt()], outs=[agh_out[:].opt()],
                )
                q = t % 4
                nc.sync.dma_start(
                    hquad[:, q * R * B:(q + 1) * R * B].rearrange("p (j b) -> p j b", j=R),
                    agh_out[:].rearrange("(j p) b -> p j b", p=P).bitcast(F32R),
                )

                # bf16 copy for pred (state s = t+1, pair pos = t % 2)
                if t % 2 == 0:
                    sbf = pairs.tile([P, 2 * R * B], BF16, tag="pairbuf", name=f"pair_{t}")
                else:
                    sbf = None
                return c_new, h_newT, q, sbf

            # ---- main loop -----------------------------------------------
            state_tile, state_off = h0slots, 0
            cur_pair = None
            for t in range(nsteps):
                c_new, h_newT, q, new_pair = step(t, state_tile, state_off)
                if new_pair is not None:
                    cur_pair = new_pair
                # copy state_{t+1} bf16 into pair position t%2
                nc.vector.tensor_copy(
                    cur_pair[:].rearrange("p (j d b) -> p j d b", j=R, d=2)[:, :, t % 2, :],
                    hquad[:, q * R * B:(q + 1) * R * B].rearrange("p (j b) -> p j b", j=R).bitcast(F32),
                )
                state_tile, state_off = hquad, q * R * B

                if t % 2 == 1:
                    # pair (t-1, t) complete -> enqueue pred chunks
                    pair_tile = cur_pair
                    p_idx = t // 2
                    for nchk in range(NCH):
                        def emit(pair_tile=pair_tile, p_idx=p_idx, nchk=nchk):
                            pps = ps_pred.tile([P, VCH], F32, tag="predps")
                            for j in range(KT_H):
                                nc.tensor.matmul(
                                    pps[:],
                                    pair_tile[:, j * 2 * B:(j + 1) * 2 * B],
                                    wp[:, j * VS + nchk * VCH: j * VS + (nchk + 1) * VCH],
                                    start=(j == 0), stop=(j == KT_H - 1),
                                    skip_group_check=True,
                                )
                            stg = stage.tile([P, VCH], F32, tag="predstage")
                            nc.vector.tensor_tensor(
                                stg[:], pps[:], bp[:, nchk * VCH:(nchk + 1) * VCH],
                                mybir.AluOpType.add,
                            )
                            for d in range(2):
                                nc.sync.dma_start(
                                    preds_out[:, 2 * p_idx + d, nchk * VCH:(nchk + 1) * VCH],
                                    stg[d * B:(d + 1) * B, :],
                                )
                        pending_preds.append(emit)

                if t == nsteps - 1:
                    nc.sync.dma_start(h_out[:], h_newT[:])
                    nc.sync.dma_start(c_out[:], c_new[:])

            emit_pred_chunks(len(pending_preds))

    nc.finalize()
    return nc


# ---------------------------------------------------------------------------
# Host side
# ---------------------------------------------------------------------------

def _host_prep(inputs):
    ctx = np.asarray(inputs["context"], np.float32)      # [B, S, C]
    data = np.asarray(inputs["data"])                    # [B, T]
    h0 = np.asarray(inputs["h0"], np.float32)            # [B, H]
    c0 = np.asarray(inputs["c0"], np.float32)
    Wk = np.asarray(inputs["Wk"], np.float32)            # [H, C]
    bk = np.asarray(inputs["bk"], np.float32)
    Wv = np.asarray(inputs["Wv"], np.float32)            # [H, C]
    bv = np.asarray(inputs["bv"], np.float32)
    embed = np.asarray(inputs["embed"], np.float32)      # [V, E]
    W_ih = np.asarray(inputs["W_ih"], np.float32)        # [4H, E+H]
    W_hh = np.asarray(inputs["W_hh"], np.float32)        # [4H, H]
    b_ih = np.asarray(inputs["b_ih"], np.float32)
    b_hh = np.asarray(inputs["b_hh"], np.float32)
    Wp = np.asarray(inputs["Wp"], np.float32)            # [V, H]
    bp = np.asarray(inputs["bp"], np.float32)

    W_ih_emb = W_ih[:, :E]                               # [4H, E]
    W_ih_val = W_ih[:, E:]                               # [4H, H]
    W_fold = W_ih_val @ Wv                               # [4H, C]
    b_comb = b_ih + b_hh + W_ih_val @ bv                 # [4H]
    Kmat = np.concatenate([W_ih_emb.T, W_fold.T, W_hh.T], axis=0)  # [2048, 4H]

    emb_all = embed[data]                                # [B, T, E]
    embT = np.ascontiguousarray(
        emb_all.transpose(2, 1, 0).reshape(KT_E, P, T, B).transpose(1, 2, 0, 3)
    ).reshape(P, T * KT_E * B)

    h0slots = np.ascontiguousarray(
        h0.T.reshape(R, P, B).transpose(1, 0, 2)
    ).reshape(P, R * B)

    bk_t = np.ascontiguousarray(bk.reshape(KT_H, P).T)

    mask = np.full((BL, BL * S), -1e30, np.float32)
    for b in range(BL):
        mask[b, b * S:(b + 1) * S] = 0.0

    in_maps = []
    for r in range(R):
        sl = slice(BL * r, BL * (r + 1))
        ctx_r = ctx[sl]                                  # [8, S, C]
        ctxT = np.ascontiguousarray(
            ctx_r.transpose(2, 0, 1).reshape(KT_C, P, BL * S).transpose(1, 0, 2)
        ).reshape(P, KT_C * BL * S)
        ctxflat = np.ascontiguousarray(
            ctx_r.transpose(1, 0, 2).reshape(P, BL * C)
        ).astype(ml_dtypes.bfloat16)
        wkT = np.ascontiguousarray(
            Wk.T.reshape(KT_C, P, H).transpose(1, 0, 2)
        ).reshape(P, KT_C * H)

        g_idx = np.concatenate([np.arange(HS) + g * H + HS * r for g in range(4)])
        wcomb = np.ascontiguousarray(
            Kmat[:, g_idx].reshape(16, P, G).transpose(1, 0, 2)
        ).reshape(P, 16 * G)
        bcomb = np.tile(b_comb[g_idx][None, :], (B, 1)).astype(np.float32)

        vsl = slice(VS * r, VS * (r + 1))
        wp_r = np.ascontiguousarray(
            Wp[vsl].T.reshape(KT_H, P, VS).transpose(1, 0, 2)
        ).reshape(P, KT_H * VS).astype(ml_dtypes.bfloat16)
        bp_r = np.tile(bp[vsl][None, :], (P, 1)).astype(ml_dtypes.bfloat16)

        hpack0 = np.ascontiguousarray(
            h0slots.reshape(P, R, B)[:, :, BL * r: BL * (r + 1)]
        ).reshape(P, R * BL)

        in_maps.append({
            "ctxT": ctxT,
            "ctxflat": ctxflat,
            "wkT": wkT,
            "bk": bk_t,
            "wcomb": wcomb,
            "bcomb": bcomb,
            "wp": wp_r,
            "bp": bp_r,
            "embT": embT,
            "h0slots": h0slots,
            "hpack0": hpack0,
            "c0T": np.ascontiguousarray(c0[:, HS * r: HS * (r + 1)]),
            "id64": np.eye(B, dtype=np.float32),
            "id8": np.eye(BL, dtype=np.float32),
            "id8b": np.eye(BL, dtype=ml_dtypes.bfloat16),
            "mask": mask.astype(ml_dtypes.bfloat16),
        })
    return in_maps


_CACHE = {}


def _get_runner():
    if "runner" in _CACHE:
        return _CACHE["runner"]
    import jax
    from jax.sharding import Mesh, PartitionSpec
    from jax.experimental.shard_map import shard_map
    from concourse.bass2jax import _bass_exec_p, install_neuronx_cc_hook, partition_id_tensor

    nc = build_nc()
    install_neuronx_cc_hook()
    partition_name = nc.partition_id_tensor.name if nc.partition_id_tensor else None

    in_names, out_names, out_avals, zero_outs = [], [], [], []
    for alloc in nc.m.functions[0].allocations:
        if not isinstance(alloc, mybir.MemoryLocationSet):
            continue
        name = alloc.memorylocations[0].name
        if alloc.kind == "ExternalInput":
            if name != partition_name:
                in_names.append(name)
        elif alloc.kind == "ExternalOutput":
            out_names.append(name)
            shape = tuple(alloc.tensor_shape)
            dtype = mybir.dt.np(alloc.dtype)
            out_avals.append(jax.core.ShapedArray(shape, dtype))
            zero_outs.append(np.zeros(shape, dtype))
    n_params = len(in_names)
    n_outs = len(out_avals)
    all_in_names = list(in_names) + list(out_names)
    if partition_name is not None:
        all_in_names.append(partition_name)

    def _body(*args):
        operands = list(args)
        if partition_name is not None:
            operands.append(partition_id_tensor())
        outs = _bass_exec_p.bind(
            *operands,
            out_avals=tuple(out_avals),
            in_names=tuple(all_in_names),
            out_names=tuple(out_names),
            lowering_input_output_aliases=(),
            sim_require_finite=True,
            sim_require_nnan=True,
            nc=nc,
        )
        return tuple(outs)

    devices = jax.devices()[:R]
    mesh = Mesh(np.asarray(devices), ("core",))
    in_specs = (PartitionSpec("core"),) * (n_params + n_outs)
    out_specs = (PartitionSpec("core"),) * n_outs
    fn = jax.jit(
        shard_map(_body, mesh=mesh, in_specs=in_specs, out_specs=out_specs, check_rep=False),
        keep_unused=True,
    )
    sharding = jax.sharding.NamedSharding(mesh, PartitionSpec("core"))

    def put(in_maps):
        arrs = [
            jax.device_put(
                np.concatenate([np.asarray(m[name]) for m in in_maps], axis=0), sharding
            )
            for name in in_names
        ]
        zeros = [
            jax.device_put(np.zeros((R * z.shape[0], *z.shape[1:]), z.dtype), sharding)
            for z in zero_outs
        ]
        return arrs, zeros

    def exec_only(arrs, zeros):
        return fn(*arrs, *zeros)

    def fetch(out_arrs):
        res = []
        for c in range(R):
            d = {}
            for i, name in enumerate(out_names):
                full = np.asarray(out_arrs[i])
                d[name] = full.reshape(R, *out_avals[i].shape)[c]
            res.append(d)
        return res

    def run(in_maps):
        arrs, zeros = put(in_maps)
        return fetch(exec_only(arrs, zeros))

    run.put = put
    run.exec_only = exec_only
    run.fetch = fetch
    _CACHE["runner"] = run
    return run


def kernel(**inputs):
    in_maps = _host_prep(inputs)
    run = _get_runner()
    res = run(in_maps)
    preds = np.concatenate([res[r]["preds"] for r in range(R)], axis=2)
    hT = np.concatenate([res[r]["h_fin"] for r in range(R)], axis=1)
    cT = np.concatenate([res[r]["c_fin"] for r in range(R)], axis=1)
    return preds, hT, cT
